# revision 8
# baseline (speedup 1.0000x reference)
"""Multi-head causal attention on 8 Trainium2 NeuronCores.

Sharding: 8 cores = 4 batches x 2 head-halves.  Each core computes, for one
batch, 8 of the 16 heads end-to-end (QKV projection with column-sharded
weights, causal attention, and a partial output projection with row-sharded
Wo).  The host sums the two partial outputs per batch and adds bo.

All matmuls run in float32r (full PE rate, ~1e-4 matmul rel err).  Host
pre-transposes the activations so no on-chip transposes are needed:

  qT/kT  [feat, seq]   = (W[:, cols].T @ X.T) accumulated over D chunks
  scoresT[k, q]        = kT_h.T-slice vs qT_h-slice matmul (K = DH = 64)
  expT                 = ACT exp (scale=1/sqrt(DH)) straight off PSUM
  ctxT_aug[65, q]      = v_aug.T @ expT  (col 64 of v_aug is ones -> row 64
                         of ctxT is the softmax denominator)
  out[q, :]            = sum over feat chunks of ctxT-normalized.T @ Wo
"""

import sys

if "/opt/trn_rl_repo" not in sys.path:
    sys.path.insert(0, "/opt/trn_rl_repo")

import numpy as np

import concourse.bass as bass
import concourse.tile as tile
from concourse import bacc, mybir
from concourse.masks import make_causal_mask  # noqa: F401  (affine_select ref)

F32 = mybir.dt.float32
F32R = mybir.dt.float32r

# Problem shape (hardcoded per the harness contract).
B, S, D, H = 4, 2048, 1024, 16
DH = D // H
N_CORES = 8
HC = H // 2              # heads per core
DC = HC * DH             # feature columns per core (512)
QB = 512                 # query block (free dim of scoresT matmuls)
KB = 128                 # key block (partition dim of scoresT)
RB = 256                 # row block for projections (moving free dim)


def _build_core_kernel(mask_mode: str, mm_dt=F32R):
    """mask_mode: 'causal' | 'dense' | 'general'."""
    nc = bacc.Bacc("TRN2", target_bir_lowering=False, debug=False)

    xq_d = nc.dram_tensor("xqT", [D, S], F32, kind="ExternalInput")
    xk_d = nc.dram_tensor("xkT", [D, S], F32, kind="ExternalInput")
    xv_d = nc.dram_tensor("xvT", [D, S], F32, kind="ExternalInput")
    wq_d = nc.dram_tensor("wq", [D, DC], F32, kind="ExternalInput")
    wk_d = nc.dram_tensor("wk", [D, DC], F32, kind="ExternalInput")
    wv_d = nc.dram_tensor("wv", [D, DC], F32, kind="ExternalInput")
    wo_d = nc.dram_tensor("wo", [DC, D], F32, kind="ExternalInput")
    if mask_mode == "general":
        # host passes mask[0,0].T * -1e9, shape [S(k), S(q)]
        mneg_d = nc.dram_tensor("maskTneg", [S, S], F32, kind="ExternalInput")
    out_d = nc.dram_tensor("out", [S, D], F32, kind="ExternalOutput")

    n_kc = D // 128          # contraction chunks for projections (8)
    n_ch = DC // 128         # feature chunks per core (4); 2 heads per chunk
    n_rb = S // RB           # projection row blocks
    n_qb = S // QB           # query blocks (4)
    n_kb = S // KB           # key blocks (16)
    kb_per_qb = QB // KB     # diag kblocks per query block (4)

    inv_sqrt_dh = 1.0 / float(np.sqrt(DH))

    with tile.TileContext(nc) as tc:
        with (
            tc.tile_pool(name="res", bufs=1) as res,
            tc.tile_pool(name="small", bufs=1) as small,
        ):
            # ---- constants -------------------------------------------------
            tri = small.tile([KB, KB], F32, tag="tri")
            nc.gpsimd.memset(tri[:], 0.0)
            ones_c = small.tile([128, HC], F32, tag="ones_c")
            nc.gpsimd.memset(ones_c[:], 1.0)
            if mask_mode != "dense":
                # scoresT[k, q]: keep where q - k >= 0, else -1e9
                nc.gpsimd.affine_select(
                    out=tri[:], in_=tri[:],
                    compare_op=mybir.AluOpType.is_ge,
                    fill=-1e9, base=0,
                    pattern=[[1, KB]], channel_multiplier=-1,
                )

            # ---- resident tensors -----------------------------------------
            qT = [res.tile([128, S], mm_dt, tag=f"qT{c}", name=f"qT{c}")
                  for c in range(n_ch)]
            kT = [res.tile([128, S], mm_dt, tag=f"kT{c}", name=f"kT{c}")
                  for c in range(n_ch)]
            ctxT = [res.tile([128, S], mm_dt, tag=f"ctxT{c}", name=f"ctxT{c}")
                    for c in range(n_ch)]
            # v_aug: per 128-row block, per head, 64 value cols + ones col
            v_aug = [res.tile([128, HC, DH + 1], mm_dt, tag=f"v{r}", name=f"v{r}")
                     for r in range(n_kb)]

            # ---- phase A0: V projection -----------------------------------
            with (
                tc.tile_pool(name="wvp", bufs=1) as wvp,
                tc.tile_pool(name="xvp", bufs=3) as xvp,
                tc.tile_pool(name="pps", bufs=3, space="PSUM") as pps,
            ):
                wv_t = wvp.tile([128, n_kc, DC], mm_dt, tag="wv")
                nc.gpsimd.dma_start(
                    wv_t[:], wv_d.ap().rearrange("(c p) n -> p c n", p=128))
                for r in range(n_kb):
                    xv_t = xvp.tile([128, n_kc, KB], mm_dt, tag="xv")
                    nc.gpsimd.dma_start(
                        xv_t[:],
                        xv_d.ap().rearrange("(c p) s -> p c s", p=128)
                        [:, :, r * KB:(r + 1) * KB])
                    ps = pps.tile([128, DC], F32, tag="pv")
                    for kc in range(n_kc):
                        nc.tensor.matmul(
                            ps[:], xv_t[:, kc, :], wv_t[:, kc, :],
                            start=(kc == 0), stop=(kc == n_kc - 1))
                    # ones column for the softmax denominator
                    nc.vector.tensor_copy(v_aug[r][:, :, DH], ones_c[:])
                    # strided copy psum [128, HC*DH] -> v_aug[:, :, 0:DH]
                    nc.scalar.activation(
                        v_aug[r][:, :, 0:DH],
                        ps[:].rearrange("p (h d) -> p h d", h=HC),
                        mybir.ActivationFunctionType.Copy)

            # ---- phase A1: Q/K projections --------------------------------
            with (
                tc.tile_pool(name="wqk", bufs=1) as wqk,
                tc.tile_pool(name="xqk", bufs=3) as xqk,
                tc.tile_pool(name="pqk", bufs=4, space="PSUM") as pqk,
            ):
                wq_t = wqk.tile([128, n_kc, DC], mm_dt, tag="wq")
                wk_t = wqk.tile([128, n_kc, DC], mm_dt, tag="wk")
                nc.gpsimd.dma_start(
                    wq_t[:], wq_d.ap().rearrange("(c p) n -> p c n", p=128))
                nc.gpsimd.dma_start(
                    wk_t[:], wk_d.ap().rearrange("(c p) n -> p c n", p=128))
                for r in range(n_rb):
                    xq_t = xqk.tile([128, n_kc, RB], mm_dt, tag="x")
                    xk_t = xqk.tile([128, n_kc, RB], mm_dt, tag="x")
                    nc.gpsimd.dma_start(
                        xq_t[:],
                        xq_d.ap().rearrange("(c p) s -> p c s", p=128)
                        [:, :, r * RB:(r + 1) * RB])
                    nc.gpsimd.dma_start(
                        xk_t[:],
                        xk_d.ap().rearrange("(c p) s -> p c s", p=128)
                        [:, :, r * RB:(r + 1) * RB])
                    for c in range(n_ch):
                        psq = pqk.tile([128, RB], F32, tag="pq")
                        psk = pqk.tile([128, RB], F32, tag="pk")
                        for kc in range(n_kc):
                            nc.tensor.matmul(
                                psq[:], wq_t[:, kc, c * 128:(c + 1) * 128],
                                xq_t[:, kc, :],
                                start=(kc == 0), stop=(kc == n_kc - 1))
                        for kc in range(n_kc):
                            nc.tensor.matmul(
                                psk[:], wk_t[:, kc, c * 128:(c + 1) * 128],
                                xk_t[:, kc, :],
                                start=(kc == 0), stop=(kc == n_kc - 1))
                        nc.scalar.activation(
                            qT[c][:, r * RB:(r + 1) * RB], psq[:],
                            mybir.ActivationFunctionType.Copy)
                        nc.scalar.activation(
                            kT[c][:, r * RB:(r + 1) * RB], psk[:],
                            mybir.ActivationFunctionType.Copy)

            # ---- phase B: attention ---------------------------------------
            with (
                tc.tile_pool(name="bex", bufs=3) as bex,
                tc.tile_pool(name="bse", bufs=4) as bse,
                tc.tile_pool(name="bps", bufs=3, space="PSUM") as bps,
                tc.tile_pool(name="bctx", bufs=2, space="PSUM") as bctx,
            ):
                for c in range(n_ch):
                    for half in range(2):
                        h = 2 * c + half
                        base = half * 64
                        kT_h = kT[c][base:base + 64, :]
                        qT_h = qT[c][base:base + 64, :]
                        for qb in range(n_qb):
                            q0 = qb * QB
                            if mask_mode == "causal":
                                kmax = (qb + 1) * kb_per_qb
                            else:
                                kmax = n_kb
                            psc = bctx.tile([DH + 1, QB], F32, tag="pctx")
                            # process kblocks in pairs sharing one psum tile
                            for g in range((kmax + 1) // 2):
                                kbs = [kb for kb in (2 * g, 2 * g + 1)
                                       if kb < kmax]
                                pss = bps.tile([128, 2 * QB], F32, tag="ps")
                                ext = bex.tile([128, 2 * QB], mm_dt, tag="ex")
                                for i, kb in enumerate(kbs):
                                    # j >= 0 only for diagonal kblocks
                                    j = kb - qb * kb_per_qb \
                                        if (mask_mode == "causal"
                                            and kb >= qb * kb_per_qb) else -1
                                    off = j * KB if j > 0 else 0
                                    n = QB - off
                                    nc.tensor.matmul(
                                        pss[:, i * QB + off:(i + 1) * QB],
                                        kT_h[:, kb * KB:(kb + 1) * KB],
                                        qT_h[:, q0 + off:q0 + QB],
                                        start=True, stop=True)
                                    if j >= 0:
                                        # mask the diagonal 128x128 sub-block
                                        nc.vector.tensor_tensor(
                                            pss[:, i * QB + off:
                                                i * QB + off + KB],
                                            pss[:, i * QB + off:
                                                i * QB + off + KB],
                                            tri[:], op=mybir.AluOpType.add)
                                    if mask_mode == "general":
                                        mng = bse.tile([128, QB], F32,
                                                       tag="mng")
                                        nc.sync.dma_start(
                                            mng[:],
                                            mneg_d.ap()
                                            [kb * KB:(kb + 1) * KB,
                                             q0:q0 + QB])
                                        nc.vector.tensor_tensor(
                                            pss[:, i * QB:(i + 1) * QB],
                                            pss[:, i * QB:(i + 1) * QB],
                                            mng[:], op=mybir.AluOpType.add)
                                # exp
                                if len(kbs) == 2 and all(
                                        (mask_mode != "causal")
                                        or (kb < qb * kb_per_qb)
                                        for kb in kbs):
                                    nc.scalar.activation(
                                        ext[:], pss[:],
                                        mybir.ActivationFunctionType.Exp,
                                        scale=inv_sqrt_dh)
                                else:
                                    for i, kb in enumerate(kbs):
                                        j = kb - qb * kb_per_qb \
                                            if (mask_mode == "causal"
                                                and kb >= qb * kb_per_qb) \
                                            else -1
                                        off = j * KB if j > 0 else 0
                                        nc.scalar.activation(
                                            ext[:, i * QB + off:
                                                (i + 1) * QB],
                                            pss[:, i * QB + off:
                                                (i + 1) * QB],
                                            mybir.ActivationFunctionType.Exp,
                                            scale=inv_sqrt_dh)
                                # ctx accumulation
                                for i, kb in enumerate(kbs):
                                    j = kb - qb * kb_per_qb \
                                        if (mask_mode == "causal"
                                            and kb >= qb * kb_per_qb) else -1
                                    off = j * KB if j > 0 else 0
                                    nc.tensor.matmul(
                                        psc[:, off:QB],
                                        v_aug[kb][:, h, :],
                                        ext[:, i * QB + off:(i + 1) * QB],
                                        start=(kb == 0),
                                        stop=(kb == kmax - 1))
                            # normalize: ctxT = psc[0:64] * (1/psc[64]) bcast
                            se_r = bse.tile([1, QB], F32, tag="ser")
                            se_b = bse.tile([64, QB], F32, tag="seb")
                            nc.vector.reciprocal(se_r[:], psc[DH:DH + 1, :])
                            nc.gpsimd.partition_broadcast(se_b[:], se_r[:])
                            nc.vector.tensor_tensor(
                                ctxT[c][base:base + 64, q0:q0 + QB],
                                psc[0:DH, :], se_b[:],
                                op=mybir.AluOpType.mult)

            # ---- phase C: output projection -------------------------------
            with (
                tc.tile_pool(name="cw", bufs=1) as cw,
                tc.tile_pool(name="cout", bufs=3) as cout,
                tc.tile_pool(name="cps", bufs=4, space="PSUM") as cps,
            ):
                wo_t = cw.tile([128, n_ch, D], mm_dt, tag="wo")
                nc.gpsimd.dma_start(
                    wo_t[:], wo_d.ap().rearrange("(c p) n -> p c n", p=128))
                for r in range(n_kb):          # 16 row blocks of 128 queries
                    ot = cout.tile([128, D], F32, tag="ot")
                    for oc in range(D // 512):
                        po = cps.tile([128, 512], F32, tag="po")
                        for c in range(n_ch):
                            nc.tensor.matmul(
                                po[:], ctxT[c][:, r * KB:(r + 1) * KB],
                                wo_t[:, c, oc * 512:(oc + 1) * 512],
                                start=(c == 0), stop=(c == n_ch - 1))
                        nc.scalar.activation(
                            ot[:, oc * 512:(oc + 1) * 512], po[:],
                            mybir.ActivationFunctionType.Copy)
                    nc.sync.dma_start(
                        out_d.ap()[r * KB:(r + 1) * KB, :], ot[:])

    nc.compile()
    return nc


_KERNEL_CACHE = {}


def _get_kernel(mask_mode):
    if mask_mode not in _KERNEL_CACHE:
        _KERNEL_CACHE[mask_mode] = _build_core_kernel(mask_mode)
    return _KERNEL_CACHE[mask_mode]


def _classify_mask(mask):
    m = np.asarray(mask).reshape(S, S)
    if not m.any():
        return "dense"
    iu = np.triu_indices(S, 1)
    causal = np.zeros((S, S), np.float32)
    causal[iu] = 1.0
    if np.array_equal(m, causal):
        return "causal"
    return "general"


def make_in_maps(queries, keys, values, mask, Wq, bq, Wk, bk, Wv, bv, Wo, bo):
    mask_mode = _classify_mask(mask)
    assert not np.any(bq) and not np.any(bk) and not np.any(bv), (
        "nonzero qkv biases not supported by this kernel build")
    in_maps = []
    for core in range(N_CORES):
        b, half = divmod(core, 2)
        cols = slice(half * DC, (half + 1) * DC)
        m = {
            "xqT": np.ascontiguousarray(queries[b].T),
            "xkT": np.ascontiguousarray(keys[b].T),
            "xvT": np.ascontiguousarray(values[b].T),
            "wq": np.ascontiguousarray(Wq[:, cols]),
            "wk": np.ascontiguousarray(Wk[:, cols]),
            "wv": np.ascontiguousarray(Wv[:, cols]),
            "wo": np.ascontiguousarray(Wo[half * DC:(half + 1) * DC, :]),
        }
        if mask_mode == "general":
            m["maskTneg"] = np.ascontiguousarray(
                np.asarray(mask).reshape(S, S).T * np.float32(-1e9))
        in_maps.append(m)
    return mask_mode, in_maps


def combine_results(results, bo):
    out = np.empty((B, S, D), np.float32)
    for b in range(B):
        out[b] = results[2 * b]["out"] + results[2 * b + 1]["out"]
    out += bo.reshape(1, 1, D).astype(np.float32)
    return out


def kernel(queries, keys, values, mask, Wq, bq, Wk, bk, Wv, bv, Wo, bo):
    from concourse import bass_utils

    mask_mode, in_maps = make_in_maps(
        queries, keys, values, mask, Wq, bq, Wk, bk, Wv, bv, Wo, bo)
    nc = _get_kernel(mask_mode)
    res = bass_utils.run_bass_kernel_spmd(
        nc, in_maps, core_ids=list(range(N_CORES)), trace=False)
    return combine_results(res.results, np.asarray(bo))
